# revision 1
# baseline (speedup 1.0000x reference)
"""EnhancedMultiHeadAttention on 8 Trainium2 NeuronCores (Bass/Tile).

Sharding: core c -> batch b = c//4, head group g = c%4 (4 heads of 16).
Per core, everything is computed in "transposed" layout [feature, token]:
  - LayerNorm stats via ones-matmul column sums of x and x^2 (PE), then
    normalize xT in place with broadcast mu/rstd rows (K=1 ones matmuls).
  - Fused projections q/k/gate in [feat, tok] layout; v in [tok, feat]
    layout (lhsT = zT) augmented with a ones column per head so the
    attention AV matmul also produces the softmax denominator.
  - Scores^T via lhsT=kT slice, rhs=qT slice (K=HD=64, head pairs packed
    into PE row groups 0-63/64-127); softmax over k is a plain exp (no
    max subtraction; scores are provably small for this model) and the
    denominator lands in ctx PSUM row 64 via the V ones column.
  - ctx rows are normalized by 1/denominator (broadcast via K=1 matmul),
    AllGather'd across the 4 cores of the batch group per q-block, then
    out = ctx_all @ w_out[:, cols] + b_out, gated and residual-added.
All LayerNorm gamma/beta and the 1/sqrt(HD) scale are folded into the
weights/biases on the host. sigmoid and rsqrt are computed via exp/ln so
the whole kernel uses one ACT table set (natural_log_exp_and_others).
"""

import contextlib
import os

import numpy as np

import jax

jax.config.update("jax_compilation_cache_dir", os.path.expanduser("~/.bass_jax_cache"))
jax.config.update("jax_persistent_cache_min_compile_time_secs", 0.0)
jax.config.update("jax_persistent_cache_min_entry_size_bytes", 0)

import concourse.bass as bass
import concourse.bacc as bacc
import concourse.tile as tile
from concourse import mybir
from concourse.bass_utils import run_bass_kernel_spmd
from concourse.hw_specs import get_activation_tables as _orig_gat


def _patched_gat(arch):
    # Steer the greedy ACT-table chooser to the combined ln+exp set so the
    # kernel needs exactly one table load instead of thrashing between
    # exp_and_others and natural_log every block (~2.7us per reload).
    tabs = {k: set(v) for k, v in _orig_gat(arch).items()}
    _AF = mybir.ActivationFunctionType
    for nm in ("exp_and_others", "exp_and_friends"):
        if nm in tabs:
            tabs[nm].discard(_AF.Exp)
    if "natural_log" in tabs:
        tabs["natural_log"].discard(_AF.Ln)
    return tabs


bacc.get_activation_tables = _patched_gat

B, S, D, H, HD = 2, 2048, 1024, 16, 64
NCORES = 8
GROUPS = [[0, 1, 2, 3], [4, 5, 6, 7]]
TB = 512  # token block
NB = S // TB  # 4
DC = D // 128  # 8 K-chunks
FH = 4  # heads per core
FQ = FH * HD  # 256 feature columns per core
FP = mybir.dt.float32
FR = mybir.dt.float32r  # TF32-like: 4x matmul throughput vs fp32
F16 = mybir.dt.float16  # halves AllGather bytes; ~5e-4 elementwise rounding
AF = mybir.ActivationFunctionType
EPS = 1e-5

_NC_CACHE = {}


def _bcast_ap(handle, parts):
    ap = handle.ap()
    return bass.AP(
        tensor=ap.tensor,
        offset=ap.offset,
        ap=[[0, parts]] + [list(p) for p in ap.ap],
    )


def _body(tc, t):
    nc = tc.nc
    stack = contextlib.ExitStack()
    stack.enter_context(
        nc.allow_low_precision(reason="fp32r/fp16 rounding is intentional; all matmul accumulation stays fp32 in PSUM")
    )
    pool = lambda name, bufs, space="SBUF": stack.enter_context(
        tc.tile_pool(name=name, bufs=bufs, space=space)
    )

    consts = pool("consts", 1)
    singles = pool("singles", 1)
    dramp = pool("dramp", 2, "DRAM")

    # PSUM pools (8 banks): sc 2x[128,1024]=4 | ctx 4x[65,512]=4.
    # Phases A and C borrow the tags while B is not using them.
    ps_sc = pool("ps_sc", 2, "PSUM")    # scores (double-wide) + stats x^2
    ps_ctx = pool("ps_ctx", 1, "PSUM")  # 4 tags: ctx accumulators (4 heads)

    pA_x = pool("pA_x", 2)      # [128, DC, TB] x block     32KB
    pA_sq = pool("pA_sq", 2)    # [128, TB] squares          4KB
    pA_rows = pool("pA_rows", 2)  # [1, TB] msq/var/lnv      ~8KB
    pA_ge = pool("pA_ge", 1)    # [128, TB] gate tmp         2KB
    pA_vt = pool("pA_vt", 2)    # [128, FQ] v evac tmp       2KB
    pB_pr = pool("pB_pr", 4)    # [128, 2*TB] f16 probs      8KB
    pB_rows = pool("pB_rows", 2)  # [1, TB] recip denom      2KB
    pB_bcs = pool("pB_bcs", 2)  # [64, TB] denom bcast       4KB
    pB_ctxT = pool("pB_ctxT", 2)  # [64, FH, TB] f16 ctx     8KB
    pC_ca = pool("pC_ca", 2)    # [128, 2, TB] f16 ctx_all   4KB
    pC_xr = pool("pC_xr", 1)    # [128, 2, TB] residual      4KB
    pC_osb = pool("pC_osb", 2)  # [128, TB] out staging      4KB

    # constants (fp32r tiles can't be memset directly; stage fp32 + DVE copy)
    onesf_col = consts.tile([128, 1], FP)
    nc.vector.memset(onesf_col, 1.0)
    onesf_row = consts.tile([1, 128], FP)
    nc.vector.memset(onesf_row, 1.0)
    ones_col = consts.tile([128, 1], FR)
    nc.vector.tensor_copy(out=ones_col, in_=onesf_col)
    ones_row = consts.tile([1, 128], FR)
    nc.vector.tensor_copy(out=ones_row, in_=onesf_row)
    ones_row16 = consts.tile([1, 128], F16)
    nc.vector.tensor_copy(out=ones_row16, in_=onesf_row)
    eps_t = consts.tile([1, 1], FP)
    nc.vector.memset(eps_t, EPS)

    # resident weights (DMAs deferred until after the first x block so the
    # LN stats pipeline starts immediately; see load_weights below)
    wqkg_sb = singles.tile([128, DC, 3 * FQ], FR)
    wv_sb = singles.tile([128, DC, FQ], FR)
    wout_sb = singles.tile([128, DC, FQ], F16)
    ncs_sb = singles.tile([1, 3 * FQ], FR)
    ncsv_sb = singles.tile([1, FQ], FR)

    def load_weights():
        nc.gpsimd.dma_start(out=wqkg_sb, in_=t["wqkg"].ap().rearrange("(d p) f -> p d f", p=128))
        nc.gpsimd.dma_start(out=wv_sb, in_=t["wv"].ap().rearrange("(d p) f -> p d f", p=128))
        nc.gpsimd.dma_start(out=wout_sb, in_=t["wout"].ap().rearrange("(d p) f -> p d f", p=128))
        nc.gpsimd.dma_start(out=ncs_sb, in_=t["ncs"].ap().rearrange("(o f) -> o f", o=1))
        nc.gpsimd.dma_start(out=ncsv_sb, in_=t["ncsv"].ap().rearrange("(o f) -> o f", o=1))
    bqkg_sb = singles.tile([128, 6], FP)
    nc.sync.dma_start(out=bqkg_sb, in_=t["bqkg"].ap().rearrange("(m p) -> p m", p=128))
    bout_sb = singles.tile([128, 2], FP)
    nc.sync.dma_start(out=bout_sb, in_=t["bout"].ap().rearrange("(m p) -> p m", p=128))
    bv_sb = singles.tile([128, FQ], FP)
    nc.sync.dma_start(out=bv_sb, in_=_bcast_ap(t["bv"], 128))

    # resident activations + per-block LN stats
    qT = singles.tile([128, 2, S], FR)
    kT = singles.tile([128, 2, S], FR)
    gT = singles.tile([128, 2, S], FP)
    va = singles.tile([128, S // 128, FH, HD + 1], F16)  # [k-part, kc, h, 65]
    for _kc in range(S // 128):
        for _h in range(FH):
            nc.vector.tensor_copy(out=va[:, _kc, _h, HD:HD + 1], in_=onesf_col)
    pA_mu = pool("pA_mu", 2)    # [1, TB] FR mean rows (A0(i) -> A1(i))
    pA_rsb = pool("pA_rsb", 2)  # [128, TB] rstd broadcast
    pA_rsc = pool("pA_rsc", 2)  # [128, 4] rstd columns
    mus, rsbs, rscs = {}, {}, {}

    xT_r = t["xT"].ap().rearrange("(d p) tk -> p d tk", p=128)
    xres_r = t["xres"].ap().rearrange("(m p) tk -> p m tk", p=128)

    xblks = {}

    # ---------------- Phase A0: LN stats for one token block --------------
    def phase_a0(i):
        tb = slice(i * TB, (i + 1) * TB)
        xblk = pA_x.tile([128, DC, TB], FR, tag="xblk", name=f"xblk{i}")
        for d in range(DC):
            nc.gpsimd.dma_start(out=xblk[:, d, :], in_=xT_r[:, d, tb])
        xblks[i] = xblk

        psx = ps_ctx.tile([1, TB], FP, tag="ctxp0", name=f"psx{i}")
        for d in range(DC):
            nc.tensor.matmul(
                out=psx, lhsT=ones_col, rhs=xblk[:, d, :],
                start=(d == 0), stop=(d == DC - 1),
            )
        pssq = ps_sc.tile([1, TB], FP, tag="sc", name=f"pssq{i}")
        for d in range(DC):
            xsq = pA_sq.tile([128, TB], FR, tag="xsq", name=f"xsq{i}_{d}")
            nc.vector.tensor_mul(out=xsq, in0=xblk[:, d, :], in1=xblk[:, d, :])
            nc.tensor.matmul(
                out=pssq, lhsT=ones_col, rhs=xsq,
                start=(d == 0), stop=(d == DC - 1),
            )
        mu = pA_mu.tile([1, TB], FR, tag="mu", name=f"mu{i}")
        mus[i] = mu
        nc.scalar.activation(out=mu, in_=psx, func=AF.Copy, scale=1.0 / D)
        msq = pA_rows.tile([1, TB], FP, tag="msq", name=f"msq{i}")
        nc.scalar.activation(out=msq, in_=pssq, func=AF.Copy, scale=1.0 / D)
        var = pA_rows.tile([1, TB], FP, tag="var", name=f"var{i}")
        nc.vector.tensor_mul(out=var, in0=mu, in1=mu)
        nc.vector.tensor_sub(out=var, in0=msq, in1=var)
        # rstd = exp(-0.5 * ln(var + eps))  (keeps everything in one ACT table set)
        lnv = pA_rows.tile([1, TB], FP, tag="lnv", name=f"lnv{i}")
        nc.scalar.activation(out=lnv, in_=var, func=AF.Ln, bias=eps_t[0:1, :])
        rstd = pA_rows.tile([1, TB], FR, tag="rstd", name=f"rstd{i}")
        nc.scalar.activation(out=rstd, in_=lnv, func=AF.Exp, scale=-0.5)
        # broadcast rstd to all partitions (row) and as per-token columns
        rs_b = pA_rsb.tile([128, TB], FP, tag="rs_b", name=f"rsb{i}")
        rsbs[i] = rs_b
        pbc2 = ps_ctx.tile([128, TB], FP, tag="ctxp1", name=f"pbcrs{i}")
        nc.tensor.matmul(out=pbc2, lhsT=ones_row, rhs=rstd, start=True, stop=True)
        nc.scalar.activation(out=rs_b, in_=pbc2, func=AF.Copy)
        rsc = pA_rsc.tile([128, 4], FR, tag="rsc", name=f"rsc{i}")
        rscs[i] = rsc
        for a in range(4):
            nc.sync.dma_start(
                out=rsc[:, a:a + 1], in_=rstd[0:1, a * 128:(a + 1) * 128]
            )

    # ---------------- Phase A1: projections for one token block -----------
    def phase_a1(i):
        tb = slice(i * TB, (i + 1) * TB)
        xblk = xblks.pop(i)
        mu = mus.pop(i)
        rs_b = rsbs.pop(i)
        rsc = rscs.pop(i)
        # q/k/gate projections on RAW x; mean subtraction folded in as a
        # rank-1 correction (ncs = -colsum(W)); rstd applied at evacuation:
        #   W^T((x-mu)rstd) = rstd * (W^T x + ncs * mu)
        for m in range(6):
            pqk = ps_ctx.tile([128, TB], FP, tag=f"ctxp{m % 4}", name=f"pqk{i}_{m}")
            for d in range(DC):
                nc.tensor.matmul(
                    out=pqk,
                    lhsT=wqkg_sb[:, d, m * 128:(m + 1) * 128],
                    rhs=xblk[:, d, :],
                    start=(d == 0), stop=False,
                )
            nc.tensor.matmul(
                out=pqk, lhsT=ncs_sb[0:1, m * 128:(m + 1) * 128], rhs=mu,
                start=False, stop=True,
            )
            if m < 4:
                dst = qT[:, m, tb] if m < 2 else kT[:, m - 2, tb]
                nc.vector.tensor_mul(out=dst, in0=pqk, in1=rs_b)
                nc.vector.tensor_scalar_add(
                    out=dst, in0=dst, scalar1=bqkg_sb[:, m:m + 1]
                )
            else:
                # gate = sigmoid(u + b) = 1 / (1 + exp(-u - b)); bias slot holds -b
                ge = pA_ge.tile([128, TB], FP, tag="ge", name=f"ge{i}_{m}")
                nc.vector.tensor_mul(out=ge, in0=pqk, in1=rs_b)
                nc.scalar.activation(
                    out=ge, in_=ge, func=AF.Exp, scale=-1.0,
                    bias=bqkg_sb[:, m:m + 1],
                )
                nc.vector.tensor_scalar_add(out=ge, in0=ge, scalar1=1.0)
                nc.vector.reciprocal(out=gT[:, m - 4, tb], in_=ge)

        # v projection on RAW x: [tok, feat]; correction mu (x) ncsv; rstd is
        # per-partition (token) at evacuation
        for mt in range(4):
            kcg = i * 4 + mt
            pv = ps_ctx.tile([128, FQ], FP, tag=f"ctxp{(mt + 2) % 4}", name=f"pv{i}_{mt}")
            for d in range(DC):
                nc.tensor.matmul(
                    out=pv,
                    lhsT=xblk[:, d, mt * 128:(mt + 1) * 128],
                    rhs=wv_sb[:, d, :],
                    start=(d == 0), stop=False,
                )
            nc.tensor.matmul(
                out=pv, lhsT=mu[0:1, mt * 128:(mt + 1) * 128], rhs=ncsv_sb,
                start=False, stop=True,
            )
            vtmp = pA_vt.tile([128, FQ], FP, tag="vtmp", name=f"vtmp{i}_{mt}")
            nc.vector.tensor_scalar_mul(
                out=vtmp, in0=pv, scalar1=rsc[:, mt:mt + 1].bitcast(FP)
            )
            for h in range(FH):
                nc.vector.tensor_add(
                    out=va[:, kcg, h, 0:HD],
                    in0=vtmp[:, h * HD:(h + 1) * HD],
                    in1=bv_sb[:, h * HD:(h + 1) * HD],
                )

    # ------- Phase B (attention) / AG / Phase C (output) ------------------
    def phase_b(qb):
        qs = slice(qb * TB, (qb + 1) * TB)
        ctxp = [
            ps_ctx.tile([HD + 1, TB], FP, tag=f"ctxp{h}", name=f"ctxp{qb}_{h}")
            for h in range(FH)
        ]
        for kc in range(S // 128):
            for half in range(2):
                sc = ps_sc.tile([128, 2 * TB], FP, tag="sc", name=f"sc{qb}_{kc}_{half}")
                for j in range(2):
                    nc.tensor.matmul(
                        out=sc[:, j * TB:(j + 1) * TB],
                        lhsT=kT[j * 64:(j + 1) * 64, half, kc * 128:(kc + 1) * 128],
                        rhs=qT[j * 64:(j + 1) * 64, half, qs],
                        start=True, stop=True, skip_group_check=True,
                    )
                pr = pB_pr.tile([128, 2 * TB], F16, tag="pr", name=f"pr{qb}_{kc}_{half}")
                nc.scalar.activation(out=pr, in_=sc, func=AF.Exp)
                for j in range(2):
                    h = 2 * half + j
                    nc.tensor.matmul(
                        out=ctxp[h],
                        lhsT=va[:, kc, h, :],
                        rhs=pr[:, j * TB:(j + 1) * TB],
                        start=(kc == 0), stop=(kc == S // 128 - 1),
                    )
        ctxT = pB_ctxT.tile([64, FH, TB], F16, tag="ctxT", name=f"ctxT{qb}")
        for h in range(FH):
            rden = pB_rows.tile([1, TB], FR, tag="rden", name=f"rden{qb}_{h}")
            nc.vector.reciprocal(out=rden, in_=ctxp[h][HD:HD + 1, :])
            bcp = ps_sc.tile([64, TB], FP, tag="sc", name=f"bcp{qb}_{h}")
            nc.tensor.matmul(
                out=bcp, lhsT=ones_row[0:1, 0:64], rhs=rden, start=True, stop=True
            )
            bcs = pB_bcs.tile([64, TB], FP, tag="bcs", name=f"bcs{qb}_{h}")
            nc.vector.tensor_copy(out=bcs, in_=bcp)
            nc.vector.tensor_mul(
                out=ctxT[:, h, :], in0=ctxp[h][0:HD, :], in1=bcs
            )
        cin = dramp.tile([FQ, TB], F16, tag="cin", name=f"cin{qb}")
        for h in range(FH):
            nc.sync.dma_start(out=cin[h * HD:(h + 1) * HD, :], in_=ctxT[:, h, :])
        call = dramp.tile([4, FQ, TB], F16, tag="call", name=f"call{qb}")
        nc.gpsimd.collective_compute(
            "AllGather",
            mybir.AluOpType.bypass,
            replica_groups=GROUPS,
            ins=[cin.opt()],
            outs=[call.opt()],
        )
        return call

    def phase_c(qb, call):
        qs = slice(qb * TB, (qb + 1) * TB)
        xres_sb = pC_xr.tile([128, 2, TB], FP, tag="xres_sb", name=f"xres{qb}")
        nc.sync.dma_start(out=xres_sb, in_=xres_r[:, :, qs])
        cas = []
        for c8 in range(DC):
            g4, r = divmod(c8, 2)
            ca = pC_ca.tile([128, 2, TB], F16, tag="ca", name=f"ca{qb}_{c8 // 2}") \
                if c8 % 2 == 0 else cas[-1][0]
            if c8 % 2 == 0:
                cas.append((ca, c8))
            nc.sync.dma_start(out=ca[:, c8 % 2, :], in_=call[g4, r * 128:(r + 1) * 128, :])
        po = [
            ps_sc.tile([128, TB], FP, tag="sc", name=f"po{qb}_{m}")
            for m in range(2)
        ]
        for c8 in range(DC):
            ca = cas[c8 // 2][0]
            for m in range(2):
                nc.tensor.matmul(
                    out=po[m],
                    lhsT=wout_sb[:, c8, m * 128:(m + 1) * 128],
                    rhs=ca[:, c8 % 2, :],
                    start=(c8 == 0), stop=(c8 == DC - 1),
                )
        for m in range(2):
            osb = pC_osb.tile([128, TB], FP, tag="osb", name=f"osb{qb}_{m}")
            nc.vector.tensor_scalar_add(out=osb, in0=po[m], scalar1=bout_sb[:, m:m + 1])
            nc.vector.tensor_mul(out=osb, in0=osb, in1=gT[:, m, qs])
            nc.vector.tensor_add(out=osb, in0=osb, in1=xres_sb[:, m, :])
            nc.sync.dma_start(out=t["outT"].ap()[m * 128:(m + 1) * 128, qs], in_=osb)

    # software-pipelined emission: B(0) pair 0 is emitted chunk-by-chunk
    # right after the A1 block that produces its k/v tiles, hiding its exp
    # work under the (ACT-idle) projection phase
    calls = {}
    phase_a0(0)
    load_weights()
    phase_a0(1)
    phase_a1(0)
    phase_a0(2)
    phase_a1(1)
    phase_a0(3)
    phase_a1(2)
    phase_a1(3)
    calls[0] = phase_b(0)
    calls[1] = phase_b(1)
    phase_c(0, calls[0])
    calls[2] = phase_b(2)
    phase_c(1, calls[1])
    calls[3] = phase_b(3)
    phase_c(2, calls[2])
    phase_c(3, calls[3])

    stack.close()


def build_nc():
    if "nc" in _NC_CACHE:
        return _NC_CACHE["nc"]
    nc = bacc.Bacc("TRN2", target_bir_lowering=False, debug=False, num_devices=NCORES)
    t = {}
    t["xT"] = nc.dram_tensor("xT", [D, S], FP, kind="ExternalInput")
    t["xres"] = nc.dram_tensor("xres", [FQ, S], FP, kind="ExternalInput")
    t["wqkg"] = nc.dram_tensor("wqkg", [D, 3 * FQ], FP, kind="ExternalInput")
    t["wv"] = nc.dram_tensor("wv", [D, FQ], FP, kind="ExternalInput")
    t["wout"] = nc.dram_tensor("wout", [D, FQ], FP, kind="ExternalInput")
    t["bqkg"] = nc.dram_tensor("bqkg", [3 * FQ], FP, kind="ExternalInput")
    t["ncs"] = nc.dram_tensor("ncs", [3 * FQ], FP, kind="ExternalInput")
    t["ncsv"] = nc.dram_tensor("ncsv", [FQ], FP, kind="ExternalInput")
    t["bv"] = nc.dram_tensor("bv", [FQ], FP, kind="ExternalInput")
    t["bout"] = nc.dram_tensor("bout", [FQ], FP, kind="ExternalInput")
    t["outT"] = nc.dram_tensor("outT", [FQ, S], FP, kind="ExternalOutput")
    with tile.TileContext(nc) as tc:
        _body(tc, t)
    nc.finalize()
    _NC_CACHE["nc"] = nc
    return nc


def make_in_maps(x, gamma, beta, w_qkv, b_qkv, w_out, b_out, w_gate, b_gate):
    x = np.asarray(x, np.float32)
    gamma = np.asarray(gamma, np.float32)
    beta = np.asarray(beta, np.float32)
    w_qkv = np.asarray(w_qkv, np.float32)
    b_qkv = np.asarray(b_qkv, np.float32)
    w_out = np.asarray(w_out, np.float32)
    b_out = np.asarray(b_out, np.float32)
    w_gate = np.asarray(w_gate, np.float32)
    b_gate = np.asarray(b_gate, np.float32)

    scale = np.float32(1.0 / np.sqrt(HD))
    xT = [np.ascontiguousarray(x[b].T) for b in range(B)]
    in_maps = []
    for c in range(NCORES):
        b, g = divmod(c, 4)
        cols = slice(g * FQ, (g + 1) * FQ)
        wq = w_qkv[:, 0 * D:1 * D][:, cols]
        wk = w_qkv[:, 1 * D:2 * D][:, cols]
        wv = w_qkv[:, 2 * D:3 * D][:, cols]
        bq = b_qkv[0 * D:1 * D][cols]
        bk = b_qkv[1 * D:2 * D][cols]
        bv = b_qkv[2 * D:3 * D][cols]
        wg = w_gate[:, cols]
        bg = b_gate[cols]

        gfold = lambda w: gamma[:, None] * w
        bfold = lambda w, bb: bb + beta @ w

        wq_e = gfold(wq) * scale
        bq_e = bfold(wq, bq) * scale
        wk_e = gfold(wk)
        bk_e = bfold(wk, bk)
        wv_e = gfold(wv)
        bv_e = bfold(wv, bv)
        wg_e = gfold(wg)
        bg_e = -bfold(wg, bg)  # negated: used as bias of exp(-u - b)

        in_maps.append({
            "xT": xT[b],
            "xres": np.ascontiguousarray(xT[b][cols, :]),
            "wqkg": np.ascontiguousarray(
                np.concatenate([wq_e, wk_e, wg_e], axis=1).astype(np.float32)
            ),
            "ncs": -np.concatenate([wq_e, wk_e, wg_e], axis=1).sum(axis=0).astype(np.float32),
            "ncsv": -wv_e.sum(axis=0).astype(np.float32),
            "wv": np.ascontiguousarray(wv_e.astype(np.float32)),
            "wout": np.ascontiguousarray(w_out[:, cols]),
            "bqkg": np.concatenate([bq_e, bk_e, bg_e]).astype(np.float32),
            "bv": bv_e.astype(np.float32),
            "bout": np.ascontiguousarray(b_out[cols]),
        })
    return in_maps


def run_device(in_maps):
    nc = build_nc()
    return run_bass_kernel_spmd(nc, in_maps, list(range(NCORES)))


def assemble(results):
    out = np.empty((B, S, D), np.float32)
    for c in range(NCORES):
        b, g = divmod(c, 4)
        out[b][:, g * FQ:(g + 1) * FQ] = results[c]["outT"].T
    return out


def kernel(**inputs):
    in_maps = make_in_maps(**inputs)
    res = run_device(in_maps)
    return assemble(res.results)



# revision 2
# speedup vs baseline: 1.0084x; 1.0084x over previous
"""EnhancedMultiHeadAttention on 8 Trainium2 NeuronCores (Bass/Tile), v2.

Sharding: core c -> batch b = c//4, head group g = c%4 (4 heads of 16).
Everything is computed in "transposed" layout [feature, token].

Key design points vs v1 (362us):
  - fp8e4m3 DoubleRow matmuls (0.5 cyc/row, 2 K-chunks of 128 per instr)
    for the LN-stat column sums, q/k/gate/v projections, scores and AV:
    PE busy drops ~212us -> ~85us.  The host pre-quantizes x*16 and
    x^2*4 to fp8 so LayerNorm stats need no device elementwise work; the
    mean/bias are folded into the matmuls as a rank-2 correction
    (ncs x mu + b x 1/rstd).
  - The softmax exp (ACT engine, ~133us for S^2 scores: the real floor)
    is software-pipelined under everything else: scores for q-block 0
    are interleaved into the projection phase (probabilities stored as
    fp8, pr = exp(s)*8, written directly by the ACT instruction), and
    the AV/output phases of group g-1 are interleaved into group g's
    score sweep.
  - The 4x41us AllGather of ctx is replaced by out = ReduceScatter(
    ctx_own @ W_out[own rows, :]) per 512-token group (4 collectives of
    21.5us, pipelined behind the exp stream).
  - PSUM: tag "sc" [128,4,256]x2 (scores / po / bc), tag "wk" (2 banks)x2
    (A-phase pqk/pv/stats rotation, post-A ctx accumulators).
All scale factors (fp8 ranges, 1/sqrt(HD), LN rstd, softmax *8) are folded
into host-prepared weights, ACT scale/bias slots, or const lhsT rows, so
the device does no extra scaling work.
"""

import contextlib
import os

import numpy as np
import ml_dtypes

import jax

jax.config.update("jax_compilation_cache_dir", os.path.expanduser("~/.bass_jax_cache"))
jax.config.update("jax_persistent_cache_min_compile_time_secs", 0.0)
jax.config.update("jax_persistent_cache_min_entry_size_bytes", 0)

import concourse.bass as bass
import concourse.bacc as bacc
import concourse.tile as tile
from concourse import mybir
from concourse.bass_utils import run_bass_kernel_spmd
from concourse.hw_specs import get_activation_tables as _orig_gat


def _patched_gat(arch):
    # Steer the greedy ACT-table chooser to the combined ln+exp set so the
    # kernel needs exactly one table load (rstd = exp(-0.5*ln(var+eps))).
    tabs = {k: set(v) for k, v in _orig_gat(arch).items()}
    _AF = mybir.ActivationFunctionType
    for nm in ("exp_and_others", "exp_and_friends"):
        if nm in tabs:
            tabs[nm].discard(_AF.Exp)
    if "natural_log" in tabs:
        tabs["natural_log"].discard(_AF.Ln)
    return tabs


bacc.get_activation_tables = _patched_gat

B, S, D, H, HD = 2, 2048, 1024, 16, 64
NCORES = 8
GROUPS = [[0, 1, 2, 3], [4, 5, 6, 7]]
TB = 512          # token block (A phase) == q group (RS granularity)
NB = S // TB      # 4
FH = 4            # heads per core
FQ = FH * HD      # 256 feature columns per core
NKP = S // 256    # 8 k-chunk pairs (256 tokens contracted per DR matmul)
FP = mybir.dt.float32
FR = mybir.dt.float32r
F16 = mybir.dt.float16
F8 = mybir.dt.float8e4
DR = mybir.MatmulPerfMode.DoubleRow
AF = mybir.ActivationFunctionType
ALU = mybir.AluOpType
EPS = 1e-5

# fp8 scale plan (see numerics.py):
XS = 16.0    # x8 = x * 16            (|x|<5.1 -> <82)
XQS = 4.0    # xq8 = x^2 * 4          (x^2<26 -> <103)
AQ = 32.0    # wq_host = gfold(wq)*32 (folds 1/sqrt(64)*256); qT8 = 256*q
AK = 16.0    # wk_host = gfold(wk)*16; kT8 = 16*k
AG = 32.0    # wg_host = gfold(wg)*32; gu = 32*u (f16)
AV_ = 16.0   # wv_host = gfold(wv)*16; va = 16*v
ESC = 1.0 / 4096.0   # scores_psum = 4096*s
PS = 8.0             # pr = exp(s)*8 (max es ~22.5 -> 180 < 240)
EBIAS = float(np.log(PS))
GESC = -1.0 / 32.0

_NC_CACHE = {}
_BIAS_FREE = [True]


def _body(tc, t):
    nc = tc.nc
    stack = contextlib.ExitStack()
    stack.enter_context(
        nc.allow_low_precision(reason="fp8/f16 rounding is intentional; matmul accumulation stays fp32 in PSUM")
    )
    pool = lambda name, bufs, space="SBUF": stack.enter_context(
        tc.tile_pool(name=name, bufs=bufs, space=space)
    )

    consts = pool("consts", 1)
    singles = pool("singles", 1)

    # PSUM (8 banks): sc 2x(2 banks) | wk 2x(2 banks).
    ps_sc = pool("ps_sc", 2, "PSUM")
    ps_wk = pool("ps_wk", 2, "PSUM")

    # ---- consts -----------------------------------------------------------
    onesf = consts.tile([128, 2, 16], FP)
    nc.vector.memset(onesf, 1.0)
    ones8t = consts.tile([128, 2, 16], F8)
    nc.vector.tensor_copy(out=ones8t, in_=onesf)
    ones8 = ones8t[:, :, 0:1]
    crow_rsb = consts.tile([1, 128], F16)      # rs_b = rstd/XS broadcast
    nc.vector.memset(crow_rsb, 1.0 / XS)
    crow_svf = consts.tile([1, 64], FP)
    nc.vector.memset(crow_svf, 1.0 / AV_)      # ctx descale 1/16
    crow_sv = consts.tile([1, 64], FR)
    nc.vector.tensor_copy(out=crow_sv, in_=crow_svf)
    eps_t = consts.tile([1, 1], FP)
    nc.vector.memset(eps_t, EPS)
    zrow = consts.tile([1, 1], FP)
    nc.vector.memset(zrow, 0.0)
    zcol = consts.tile([128, 1], FP)
    nc.vector.memset(zcol, 0.0)
    ebias = consts.tile([128, 1], FP)
    nc.vector.memset(ebias, EBIAS)

    # ---- resident weights / inputs ---------------------------------------
    w8_sb = singles.tile([128, 4, 2, 3 * FQ], F8)
    wv8_sb = singles.tile([128, 4, 2, FQ], F8)
    wo_sb = singles.tile([64, 4, D], F16)
    cq_sb = singles.tile([2, 3 * FQ], F16)
    cv_sb = singles.tile([2, FQ], F16)
    gc_sb = singles.tile([128, 2], FP)
    bout_sb = singles.tile([128, 2], FP)
    xres_sb = singles.tile([128, 2, S], F16)

    x8_r = t["x8"].ap().rearrange("(kp c p) s -> p kp c s", p=128, c=2)
    xq8_r = t["xq8"].ap().rearrange("(kp c p) s -> p kp c s", p=128, c=2)
    xres_r = t["xres"].ap().rearrange("(m p) s -> p m s", p=128)
    outT_r = t["outT"].ap().rearrange("(m p) s -> p m s", p=128)

    def load_weights():
        g = nc.gpsimd
        g.dma_start(out=w8_sb, in_=t["w8"].ap().rearrange("(kp c p) f -> p kp c f", p=128, c=2))
        g.dma_start(out=cq_sb, in_=t["cq"].ap())
        g.dma_start(out=wv8_sb, in_=t["wv8"].ap().rearrange("(kp c p) f -> p kp c f", p=128, c=2))
        g.dma_start(out=cv_sb, in_=t["cv"].ap())

    def load_weights2():
        g = nc.gpsimd
        g.dma_start(out=wo_sb, in_=t["wo"].ap().rearrange("(kc p) f -> p kc f", p=64))
        g.dma_start(out=gc_sb, in_=t["gc"].ap().rearrange("(m p) -> p m", p=128))
        g.dma_start(out=bout_sb, in_=t["bout"].ap().rearrange("(m p) -> p m", p=128))

    # ---- per-block activations (block == q-group) -------------------------
    qdr = [singles.tile([32, 2, FH, TB], F8, name=f"qdr{i}") for i in range(NB)]
    kdr = [singles.tile([32, 2, FH, TB], F8, name=f"kdr{i}") for i in range(NB)]
    gTs = [singles.tile([128, 2, TB], F16, name=f"gT{i}") for i in range(NB)]
    HDP = 80  # 65 used + pad so the DoubleRow pair stride (4*80) is 16-aligned
    vas = [singles.tile([128, 2, 2, FH, HDP], F8, name=f"va{i}") for i in range(NB)]
    for i in range(NB):
        nc.vector.memset(vas[i], 0.0)
        nc.vector.memset(vas[i][:, :, :, :, HD:HD + 1], 1.0)  # denominator col

    p_x8 = pool("p_x8", 3)        # [128, 4, 2, TB] fp8
    p_xq8 = pool("p_xq8", 2)
    p_strows = pool("p_strows", 2)  # [2, TB] f16: row0 mu, row1 1/rstd
    p_rows = pool("p_rows", 2)    # [1, TB] rows (mu2/var/lnv/rstd)
    p_rsb = pool("p_rsb", 2)      # [128, TB] f16 rstd/XS
    p_rsc = pool("p_rsc", 2)      # [128, 4] f16 rstd cols
    p_qk8 = pool("p_qk8", 2)      # [128, 2, TB] fp8 (q and k tags)
    p_gu = pool("p_gu", 2)        # [128, 2, TB] f16
    p_pr = pool("p_pr", 20)       # [128, 2, FH, 256] fp8 probs (per k-pair, unit)
    p_rden = pool("p_rden", 2)    # [1, FH, 256] FR recip denominators
    p_ctxT = pool("p_ctxT", 2)    # [64, FH, TB] f16
    p_pout = pool("p_pout", 1)    # [128, 8, TB] f16 staging
    p_rso = pool("p_rso", 2)      # [128, 2, TB] f16 RS result
    p_fo16 = pool("p_fo16", 2)    # [128, TB] f16
    p_fo = pool("p_fo", 2)        # [128, 2, TB] f32
    dram_po = pool("dram_po", 3, "DRAM")
    dram_rs = pool("dram_rs", 4, "DRAM")

    x8s, xq8s, strowss, rsbs, rscs = {}, {}, {}, {}, {}
    prs = {}       # (g, u, kp) -> pr tile
    ctxs = {}      # (g, u) -> ctx accumulator
    ctxTs = {}     # g -> ctxT staging tile
    ctxcs = {}     # (g, u) -> unnormalized ctx copy
    parts = {}     # g -> dram partial tile
    pouts_t = {}   # g -> pout staging tile
    rsres = {}     # g -> dram RS result tile
    gts = {}       # g -> finished gate tiles

    def dma_x8(i):
        tb = slice(i * TB, (i + 1) * TB)
        x8 = p_x8.tile([128, 4, 2, TB], F8, tag="x8", name=f"x8_{i}")
        nc.sync.dma_start(out=x8, in_=x8_r[:, :, :, tb])
        x8s[i] = x8

    def dma_xq8(i):
        tb = slice(i * TB, (i + 1) * TB)
        xq = p_xq8.tile([128, 4, 2, TB], F8, tag="xq", name=f"xq{i}")
        nc.sync.dma_start(out=xq, in_=xq8_r[:, :, :, tb])
        xq8s[i] = xq

    def dma_block(i):
        dma_x8(i)
        dma_xq8(i)

    # ---------------- Phase A: LN stats for one token block ---------------
    def stats(i):
        x8 = x8s[i]
        xq = xq8s.pop(i)
        st = ps_wk.tile([1, 2, TB], FP, tag="wk", name=f"st{i}")
        for kp in range(4):
            nc.tensor.matmul(
                out=st[0:1, 0, :], lhsT=ones8, rhs=x8[:, kp, :, :],
                start=(kp == 0), stop=(kp == 3), perf_mode=DR,
            )
        for kp in range(4):
            nc.tensor.matmul(
                out=st[0:1, 1, :], lhsT=ones8, rhs=xq[:, kp, :, :],
                start=(kp == 0), stop=(kp == 3), perf_mode=DR,
            )
        # mu (f16 row, true units), var, rstd = exp(-0.5*ln(var+eps))
        strows = p_strows.tile([2, TB], F16, tag="strows", name=f"strows{i}")
        strowss[i] = strows
        nc.vector.tensor_scalar_mul(
            out=strows[0:1, :], in0=st[0:1, 0, :], scalar1=1.0 / (XS * D)
        )
        mu2 = p_rows.tile([1, TB], FP, tag="row", name=f"mu2{i}")
        nc.vector.tensor_mul(out=mu2, in0=strows[0:1, :], in1=strows[0:1, :])
        var = p_rows.tile([1, TB], FP, tag="row", name=f"var{i}")
        nc.vector.scalar_tensor_tensor(
            out=var, in0=st[0:1, 1, :], scalar=1.0 / (XQS * D), in1=mu2,
            op0=ALU.mult, op1=ALU.subtract,
        )
        lnv = p_rows.tile([1, TB], FP, tag="row", name=f"lnv{i}")
        nc.scalar.activation(out=lnv, in_=var, func=AF.Ln, bias=eps_t[0:1, :])
        rstd = p_rows.tile([1, TB], F16, tag="rstd", name=f"rstd{i}")
        nc.scalar.activation(out=rstd, in_=lnv, func=AF.Exp, scale=-0.5, bias=zrow[0:1, :])
        if not _BIAS_FREE[0]:
            # DVE ops cannot write at partition offset 1; go through a DMA hop
            invr = p_rows.tile([1, TB], F16, tag="invr", name=f"invr{i}")
            nc.vector.reciprocal(out=invr, in_=rstd)
            nc.sync.dma_start(out=strows[1:2, :], in_=invr)
        # broadcast rstd/XS to all 128 partitions
        rb = ps_sc.tile([128, TB], FP, tag="sc", name=f"rb{i}")
        nc.tensor.matmul(out=rb, lhsT=crow_rsb, rhs=rstd, start=True, stop=True)
        rs_b = p_rsb.tile([128, TB], F16, tag="rsb", name=f"rsb{i}")
        nc.vector.tensor_copy(out=rs_b, in_=rb)
        rsbs[i] = rs_b
        rsch = p_rsc.tile([128, 4], F16, tag="rsch", name=f"rsch{i}")
        for mt in range(4):
            nc.sync.dma_start(
                out=rsch[:, mt:mt + 1], in_=rstd[0:1, mt * 128:(mt + 1) * 128]
            )
        rsc = p_rsc.tile([128, 4], FP, tag="rsc", name=f"rsc{i}")
        nc.vector.tensor_copy(out=rsc, in_=rsch)
        rscs[i] = rsc

    # ---------------- Phase A: projections for one token block ------------
    def proj_qkg(i):
        x8 = x8s[i]
        strows = strowss[i]
        rs_b = rsbs[i]
        q8 = p_qk8.tile([128, 2, TB], F8, tag="q8", name=f"q8_{i}")
        k8 = p_qk8.tile([128, 2, TB], F8, tag="k8", name=f"k8_{i}")
        gus = p_gu.tile([128, 2, TB], F16, tag="gu", name=f"gu{i}")
        # q/k/gate in [feat, tok]; mean/bias folded as rank-2 correction
        for m in range(6):
            ms = slice(m * 128, (m + 1) * 128)
            pqk = ps_wk.tile([128, TB], FP, tag="wk", name=f"pqk{i}_{m}")
            for kp in range(4):
                nc.tensor.matmul(
                    out=pqk, lhsT=w8_sb[:, kp, :, ms], rhs=x8[:, kp, :, :],
                    start=(kp == 0), stop=False, perf_mode=DR,
                )
            if _BIAS_FREE[0]:
                nc.tensor.matmul(
                    out=pqk, lhsT=cq_sb[0:1, ms], rhs=strows[0:1, :],
                    start=False, stop=True, skip_group_check=True,
                )
            else:
                nc.tensor.matmul(
                    out=pqk, lhsT=cq_sb[:, ms], rhs=strows,
                    start=False, stop=True, skip_group_check=True,
                )
            if m < 2:
                nc.vector.tensor_mul(out=q8[:, m, :], in0=pqk, in1=rs_b)
                if m == 1:
                    nc.sync.dma_start(out=qdr[i][:, :, 0:4:2, :], in_=q8[0:64, :, :])
                    nc.sync.dma_start(out=qdr[i][:, :, 1:4:2, :], in_=q8[64:128, :, :])
            elif m < 4:
                nc.vector.tensor_mul(out=k8[:, m - 2, :], in0=pqk, in1=rs_b)
                if m == 3:
                    nc.sync.dma_start(out=kdr[i][:, :, 0:4:2, :], in_=k8[0:64, :, :])
                    nc.sync.dma_start(out=kdr[i][:, :, 1:4:2, :], in_=k8[64:128, :, :])
            else:
                nc.vector.tensor_mul(out=gus[:, m - 4, :], in0=pqk, in1=rs_b)
        # gate = 1 / (1 + exp(-u) * gc): only e = exp(-u) here; the cheap
        # DVE finish runs in post() where DVE is otherwise idle
        nc.scalar.activation(out=gTs[i], in_=gus, func=AF.Exp, scale=GESC, bias=zcol[:, 0:1])
    def projv(i):
        x8 = x8s.pop(i)
        strows = strowss.pop(i)
        rsbs.pop(i)
        rsc = rscs.pop(i)
        # v in [tok, feat] (lhsT = x8): va = 16*v
        for mt in range(4):
            tl = slice(mt * 128, (mt + 1) * 128)
            pv = ps_wk.tile([128, FQ], FP, tag="wk", name=f"pv{i}_{mt}")
            for kp in range(4):
                nc.tensor.matmul(
                    out=pv, lhsT=x8[:, kp, :, tl], rhs=wv8_sb[:, kp, :, :],
                    start=(kp == 0), stop=False, perf_mode=DR,
                )
            if _BIAS_FREE[0]:
                nc.tensor.matmul(
                    out=pv, lhsT=strows[0:1, tl], rhs=cv_sb[0:1, :],
                    start=False, stop=True, skip_group_check=True,
                )
            else:
                nc.tensor.matmul(
                    out=pv, lhsT=strows[:, tl], rhs=cv_sb,
                    start=False, stop=True, skip_group_check=True,
                )
            nc.vector.tensor_scalar(
                out=vas[i][:, mt // 2, mt % 2, :, 0:HD], in0=pv,
                scalar1=rsc[:, mt:mt + 1], scalar2=1.0 / XS,
                op0=ALU.mult, op1=ALU.mult,
            )


    # ------------- scores + exp for (group g, unit u, k-pair kp) -----------
    # post-A score PSUM rotates through 2 "sc" slots + 1 "wk" slot (3-deep
    # ACT backlog); during phase A only the 2 "sc" slots are used.
    scup_n = [0]

    def scexp_u(g, u, kp, in_a=False):
        pr = p_pr.tile([128, 2, FH, 256], F8, tag="pr", name=f"pr{g}_{u}_{kp}")
        prs[(g, u, kp)] = pr
        qs = slice(u * 256, (u + 1) * 256)
        for par in range(2):
            kc = 2 * kp + par
            kb, kl = divmod(kc, 4)
            ks_ = slice(kl * 128, (kl + 1) * 128)
            scup_n[0] += 1
            if in_a or scup_n[0] % 3:
                sc = ps_sc.tile([128, FH, 256], FP, tag="sc", name=f"sc{g}_{u}_{kp}_{par}")
            else:
                sc = ps_wk.tile([128, FH, 256], FP, tag="wk", name=f"sc{g}_{u}_{kp}_{par}")
            for h in range(FH):
                nc.tensor.matmul(
                    out=sc[:, h, :], lhsT=kdr[kb][:, :, h, ks_],
                    rhs=qdr[g][:, :, h, qs],
                    start=True, stop=True, perf_mode=DR, skip_group_check=True,
                )
            nc.scalar.activation(
                out=pr[:, par, :, :], in_=sc,
                func=AF.Exp, scale=ESC, bias=ebias[:, 0:1],
            )

    # ---------------- AV accumulation for (g, u, kp) ------------------------
    def alloc_ctx(g, u):
        ctxs[(g, u)] = ps_wk.tile(
            [80, FH, 256], FP, tag="wk", name=f"ctx{g}_{u}"
        )

    def av_u(g, u, kp):
        pr = prs.pop((g, u, kp))
        kb, kpl = divmod(kp, 2)
        for h in range(FH):
            nc.tensor.matmul(
                out=ctxs[(g, u)][:, h, :],
                lhsT=vas[kb][:, kpl, :, h, :],
                rhs=pr[:, :, h, :],
                start=(kp == 0), stop=(kp == NKP - 1), perf_mode=DR,
                skip_group_check=True,
            )

    # -------- evac one ctx unit (normalize to f16) + its out-proj half -----
    def evac_a(g, u):
        # unnormalized ctx copy can start as soon as AV is done (PSUM->SBUF),
        # overlapping the reciprocal/broadcast of the denominators
        ctxc = p_rden.tile([64, FH, 256], F16, tag="ctxc", name=f"ctxc{g}_{u}")
        nc.vector.tensor_copy(out=ctxc, in_=ctxs[(g, u)][0:HD, :, :])
        ctxcs[(g, u)] = ctxc

    def evac_b(g, u):
        if g not in ctxTs:
            ctxTs[g] = p_ctxT.tile([64, FH, TB], F16, tag="ctxT", name=f"ctxT{g}")
        ctxT = ctxTs[g]
        cu = ctxs.pop((g, u))
        ctxc = ctxcs.pop((g, u))
        us = slice(u * 256, (u + 1) * 256)
        rden = p_rden.tile([1, FH, 256], FR, tag="rden", name=f"rden{g}_{u}")
        for h in range(FH):
            nc.vector.reciprocal(out=rden[0:1, h, :], in_=cu[HD:HD + 1, h, :])
        bc = ps_sc.tile([64, FH, 256], FP, tag="sc", name=f"bc{g}_{u}")
        for h in range(FH):
            nc.tensor.matmul(
                out=bc[:, h, :], lhsT=crow_sv, rhs=rden[0:1, h, :],
                start=True, stop=True, skip_group_check=True,
            )
        nc.vector.tensor_mul(out=ctxT[:, :, us], in0=ctxc, in1=bc)
        if u == 1:
            ctxTs.pop(g)
        return ctxT

    ctxTs2 = {}

    def evac_u(g, u):
        evac_a(g, u)
        ctxTs2[(g, u)] = evac_b(g, u)

    def po_u(g, u, mp):
        # out-proj (K=64 chunks read ctxT directly), one pair of m-tiles
        ctxT = ctxTs2[(g, u)]
        us = slice(u * 256, (u + 1) * 256)
        if g not in parts:
            parts[g] = dram_po.tile([D, TB], F16, tag="part", name=f"part{g}")
            pouts_t[g] = p_pout.tile([128, 8, TB], F16, tag="pout", name=f"pout{g}")
        pouts = pouts_t[g]
        po = ps_sc.tile([128, 2, 256], FP, tag="sc", name=f"po{g}_{u}_{mp}")
        for m2 in range(2):
            m = 2 * mp + m2
            for h in range(FH):
                nc.tensor.matmul(
                    out=po[:, m2, :], lhsT=wo_sb[:, h, m * 128:(m + 1) * 128],
                    rhs=ctxT[:, h, us], start=(h == 0), stop=(h == FH - 1),
                    skip_group_check=True,
                )
        nc.vector.tensor_copy(out=pouts[:, 2 * mp:2 * mp + 2, us], in_=po)

    def part_half(g, u):
        ctxTs2.pop((g, u))
        us = slice(u * 256, (u + 1) * 256)
        nc.sync.dma_start(
            out=parts[g].rearrange("(m p) s -> p m s", p=128)[:, :, us],
            in_=pouts_t[g][:, :, us],
        )

    def cphase(g):
        gts[g] = []
        for m in range(2):
            gp = p_fo16.tile([128, TB], F16, tag="gp", name=f"gp{g}_{m}")
            nc.vector.tensor_scalar(
                out=gp, in0=gTs[g][:, m, :], scalar1=gc_sb[:, m:m + 1], scalar2=1.0,
                op0=ALU.mult, op1=ALU.add,
            )
            gt = p_fo16.tile([128, TB], F16, tag="gt", name=f"gt{g}_{m}")
            nc.vector.reciprocal(out=gt, in_=gp)
            gts[g].append(gt)
        parts_g = parts.pop(g)
        pouts_t.pop(g)
        rsr = dram_rs.tile([2, 128, TB], F16, tag="rsr", name=f"rsr{g}")
        rsres[g] = rsr
        nc.gpsimd.collective_compute(
            "ReduceScatter",
            ALU.add,
            replica_groups=GROUPS,
            ins=[parts_g.opt()],
            outs=[rsr.opt()],
        )

    # ---------------- post: bias + gate + residual + store -----------------
    def post(g):
        qs = slice(g * TB, (g + 1) * TB)
        rsr = rsres.pop(g)
        rso = p_rso.tile([128, 2, TB], F16, tag="rso", name=f"rso{g}")
        for m in range(2):
            nc.sync.dma_start(out=rso[:, m, :], in_=rsr[m, :, :])
        fo = p_fo.tile([128, 2, TB], FP, tag="fo", name=f"fo{g}")
        for m in range(2):
            f16t = p_fo16.tile([128, TB], F16, tag="fo16", name=f"fo16{g}_{m}")
            nc.vector.scalar_tensor_tensor(
                out=f16t, in0=rso[:, m, :], scalar=bout_sb[:, m:m + 1],
                in1=gts[g][m], op0=ALU.add, op1=ALU.mult,
            )
            nc.vector.tensor_add(out=fo[:, m, :], in0=f16t, in1=xres_sb[:, m, qs])
        gts.pop(g)
        nc.sync.dma_start(out=outT_r[:, :, qs], in_=fo)

    # ======================= emission schedule =============================
    av_next = {}
    emitted = {}   # (g, u) -> number of scexp kps emitted
    dma_x8(0)
    load_weights()
    dma_xq8(0)
    dma_block(1)
    stats(0)
    proj_qkg(0)
    for i in range(NB):
        if i + 1 < NB:
            stats(i + 1)
        for u in range(2):
            scexp_u(0, u, 2 * i, in_a=True)
        projv(i)
        for u in range(2):
            scexp_u(0, u, 2 * i + 1, in_a=True)
        if i + 1 < NB:
            proj_qkg(i + 1)
        if i + 2 < NB:
            dma_block(i + 2)
        if i == 0:
            load_weights2()
            nc.sync.dma_start(out=xres_sb, in_=xres_r)

    # post-A: one 16-exp score sweep per (group, unit); ctx accumulation of
    # the previous sweep's prs and the evac/C/RS chain are interleaved at
    # fixed kp milestones to keep the exp stream fed.
    sweeps = [(g, u) for g in range(1, NB) for u in range(2)]

    def pump(g, u, n):
        j = av_next[(g, u)]
        while j < min(av_next[(g, u)] + n, NKP):
            av_u(g, u, j)
            j += 1
        av_next[(g, u)] = j

    def start_ctx(g, u):
        alloc_ctx(g, u)
        av_next[(g, u)] = 0

    for si, (g, u) in enumerate(sweeps):
        p = ((g, u - 1) if u else (g - 1, 1))  # previous sweep
        for kp in range(NKP):
            scexp_u(g, u, kp)
            if si == 0:
                # group-0 AV/evac/out-proj (prs stored during phase A)
                if kp == 0:
                    start_ctx(0, 0)
                    pump(0, 0, 4)
                elif kp == 1:
                    pump(0, 0, 4)
                elif kp == 2:
                    evac_a(0, 0)
                elif kp == 3:
                    ctxTs2[(0, 0)] = evac_b(0, 0)
                elif kp == 4:
                    start_ctx(0, 1)
                    pump(0, 1, 4)
                    po_u(0, 0, 0)
                    po_u(0, 0, 1)
                elif kp == 5:
                    pump(0, 1, 4)
                    po_u(0, 0, 2)
                    po_u(0, 0, 3)
                    part_half(0, 0)
                elif kp == 6:
                    evac_a(0, 1)
                elif kp == 7:
                    ctxTs2[(0, 1)] = evac_b(0, 1)
            elif si == 1:
                if kp == 0:
                    start_ctx(*p)
                    pump(*p, 2)
                elif kp == 1:
                    pump(*p, NKP)
                    evac_a(*p)
                elif kp == 2:
                    ctxTs2[p] = evac_b(*p)
                elif kp == 3:
                    start_ctx(g, u)
                    pump(g, u, 1)
                    po_u(0, 1, 0)
                    po_u(0, 1, 1)
                elif kp == 4:
                    pump(g, u, 1)
                    po_u(0, 1, 2)
                    po_u(0, 1, 3)
                    part_half(0, 1)
                    cphase(0)
                elif kp == 5:
                    pump(g, u, 1)
                    po_u(*p, 0)
                elif kp == 6:
                    pump(g, u, 1)
                    po_u(*p, 1)
                    po_u(*p, 2)
                elif kp == 7:
                    pump(g, u, 1)
                    po_u(*p, 3)
                    part_half(*p)
            else:
                if kp == 0:
                    pump(*p, 2)
                elif kp == 1:
                    pump(*p, NKP)
                    evac_a(*p)
                elif kp == 2:
                    ctxTs2[p] = evac_b(*p)
                elif kp == 3:
                    start_ctx(g, u)
                    pump(g, u, 1)
                    po_u(*p, 0)
                elif kp == 4:
                    pump(g, u, 1)
                    po_u(*p, 1)
                elif kp == 5:
                    pump(g, u, 1)
                    po_u(*p, 2)
                elif kp == 6:
                    pump(g, u, 1)
                    po_u(*p, 3)
                    part_half(*p)
                    if p[1] == 1:
                        cphase(p[0])
                elif kp == 7:
                    pump(g, u, 1)
                    if p[1] == 1 and p[0] - 2 >= 0:
                        post(p[0] - 2)
    # tail: last sweep's AV remainder + finish
    g, u = sweeps[-1]
    pump(g, u, NKP)
    evac_u(g, u)
    for mp in range(4):
        po_u(g, u, mp)
    part_half(g, u)
    cphase(g)
    post(NB - 3)
    post(NB - 2)
    post(NB - 1)

    stack.close()


def build_nc():
    if "nc" in _NC_CACHE:
        return _NC_CACHE["nc"]
    nc = bacc.Bacc("TRN2", target_bir_lowering=False, debug=False, num_devices=NCORES)
    t = {}
    t["x8"] = nc.dram_tensor("x8", [D, S], F8, kind="ExternalInput")
    t["xq8"] = nc.dram_tensor("xq8", [D, S], F8, kind="ExternalInput")
    t["xres"] = nc.dram_tensor("xres", [FQ, S], F16, kind="ExternalInput")
    t["w8"] = nc.dram_tensor("w8", [D, 3 * FQ], F8, kind="ExternalInput")
    t["wv8"] = nc.dram_tensor("wv8", [D, FQ], F8, kind="ExternalInput")
    t["wo"] = nc.dram_tensor("wo", [FQ, D], F16, kind="ExternalInput")
    t["cq"] = nc.dram_tensor("cq", [2, 3 * FQ], F16, kind="ExternalInput")
    t["cv"] = nc.dram_tensor("cv", [2, FQ], F16, kind="ExternalInput")
    t["gc"] = nc.dram_tensor("gc", [FQ], FP, kind="ExternalInput")
    t["bout"] = nc.dram_tensor("bout", [FQ], FP, kind="ExternalInput")
    t["outT"] = nc.dram_tensor("outT", [FQ, S], FP, kind="ExternalOutput")
    with tile.TileContext(nc) as tc:
        _body(tc, t)
    nc.finalize()
    _NC_CACHE["nc"] = nc
    return nc


E4NP = ml_dtypes.float8_e4m3


def _q8(a):
    return np.asarray(a, np.float32).astype(E4NP)


def make_in_maps(x, gamma, beta, w_qkv, b_qkv, w_out, b_out, w_gate, b_gate):
    x = np.asarray(x, np.float32)
    gamma = np.asarray(gamma, np.float32)
    beta = np.asarray(beta, np.float32)
    w_qkv = np.asarray(w_qkv, np.float32)
    b_qkv = np.asarray(b_qkv, np.float32)
    w_out = np.asarray(w_out, np.float32)
    b_out = np.asarray(b_out, np.float32)
    w_gate = np.asarray(w_gate, np.float32)
    b_gate = np.asarray(b_gate, np.float32)

    xT = [np.ascontiguousarray(x[b].T) for b in range(B)]
    x8T = [_q8(a * XS) for a in xT]
    xq8T = [_q8((a * a) * XQS) for a in xT]

    gfold = lambda w: gamma[:, None] * w
    bfold = lambda w, bb: bb + beta @ w
    _BIAS_FREE[0] = bool(
        np.all(b_qkv == 0) and np.all(beta == 0)
    )

    in_maps = []
    for c in range(NCORES):
        b, g = divmod(c, 4)
        cols = slice(g * FQ, (g + 1) * FQ)
        wq = gfold(w_qkv[:, 0 * D:1 * D][:, cols])
        wk = gfold(w_qkv[:, 1 * D:2 * D][:, cols])
        wv = gfold(w_qkv[:, 2 * D:3 * D][:, cols])
        wg = gfold(w_gate[:, cols])
        bq = bfold(w_qkv[:, 0 * D:1 * D][:, cols], b_qkv[0 * D:1 * D][cols])
        bk = bfold(w_qkv[:, 1 * D:2 * D][:, cols], b_qkv[1 * D:2 * D][cols])
        bv = bfold(w_qkv[:, 2 * D:3 * D][:, cols], b_qkv[2 * D:3 * D][cols])
        bg = bfold(w_gate[:, cols], b_gate[cols])

        w8 = np.ascontiguousarray(np.concatenate(
            [_q8(wq * AQ), _q8(wk * AK), _q8(wg * AG)], axis=1))
        wv8 = np.ascontiguousarray(_q8(wv * AV_))
        w8f = w8.astype(np.float32)
        wv8f = wv8.astype(np.float32)
        # corr rows: row0 x mu (mean removal), row1 x (1/rstd) (bias)
        cq = np.stack([
            -XS * w8f.sum(axis=0),
            np.concatenate([512.0 * bq, 256.0 * bk, np.zeros(FQ, np.float32)]),
        ]).astype(np.float16)
        cv = np.stack([
            -XS * wv8f.sum(axis=0),
            256.0 * bv,
        ]).astype(np.float16)

        in_maps.append({
            "x8": x8T[b],
            "xq8": xq8T[b],
            "xres": xT[b][cols, :].astype(np.float16),
            "w8": w8,
            "wv8": wv8,
            "wo": np.ascontiguousarray(w_out[cols, :]).astype(np.float16),
            "cq": np.ascontiguousarray(cq),
            "cv": np.ascontiguousarray(cv),
            "gc": np.exp(-bg).astype(np.float32),
            "bout": np.ascontiguousarray(b_out[cols]).astype(np.float32),
        })
    return in_maps


def run_device(in_maps):
    nc = build_nc()
    return run_bass_kernel_spmd(nc, in_maps, list(range(NCORES)))


def assemble(results):
    out = np.empty((B, S, D), np.float32)
    for c in range(NCORES):
        b, g = divmod(c, 4)
        out[b][:, g * FQ:(g + 1) * FQ] = results[c]["outT"].T
    return out


def kernel(**inputs):
    in_maps = make_in_maps(**inputs)
    res = run_device(in_maps)
    return assemble(res.results)
